# revision 39
# baseline (speedup 1.0000x reference)
"""Trainium2 Bass kernel for DynamicTemporalAttention (ALiBi-style distance-biased MHA).

Shapes (hardcoded): x [2,2048,1024], Wq/Wk/Wv/Wo [1024,1024], biases [1024],
slopes [16].  H=16 heads, DH=64.

Sharding: 8 cores = (batch b in {0,1}) x (head-group g in {0..3}); each core
handles 4 heads of one batch.  Wq/Wk/Wv column-sharded, Wo row-sharded; the
host sums the 4 partial outputs per batch and adds bo.

The bias -softplus(slope)*|s-t| with softplus(slope) >= 0.718 makes attention
effectively banded: beyond |s-t| > 64 contributions are < 1e-20 relative, so
each 128-row query tile only attends to a 256-wide t-window (two 128 t-tiles
at offsets -64/+64, clamped at the sequence edges).

Device program per core (Tile framework):
  A) x arrives pre-transposed from the host (feature-major, bf16/fp8) - no
     on-device transposes.  Project Q^T(/8)+K^T (bf16, feature-major) and V;
     V is re-naturalized with the DMA transpose engine (no PE/DVE cost).
  B) Per 128-row s-tile: 4 heads' banded scores^T packed into one [128,512]
     PSUM tile (bf16 matmuls, k=64), one ACT exp -> bf16, one DVE multiply
     by a precomputed per-head decay strip exp(-softplus(slope)*dist)
     (host-computed, bf16, 2x DVE mode).  AV accumulates per head into a
     [128,2,128] PSUM tile (even heads rows 0:64, odd heads 64:128);
     denominators via a ones-column matmul, reciprocal on DVE, broadcast
     by a k=1 PE outer product, one fused [128,2,128] normalize-multiply.
  C) Output projection from context^T (fp8 DoubleRow or bf16), landing in
     natural [s, D] layout, bf16 DMA out (host accumulates in f32).
"""

import numpy as np
import ml_dtypes

import concourse.bass as bass
import concourse.tile as tile
from concourse import bacc
from concourse import mybir
from concourse.bass_utils import run_bass_kernel_spmd

B, S, D, H, DH = 2, 2048, 1024, 16, 64
NCORES = 8
HPC = 4           # heads per core
DPC = HPC * DH    # feature cols per core = 256
NPT = DPC // 128  # partition-tiles of the per-core feature dim = 2
KT = D // 128     # 8 contraction tiles for projections
NST = S // 128    # 16 query tiles of 128 rows
F32 = mybir.dt.float32
F32R = mybir.dt.float32r
BF16 = mybir.dt.bfloat16
FP8 = mybir.dt.float8e4
AF = mybir.ActivationFunctionType
ALU = mybir.AluOpType
DR = mybir.MatmulPerfMode.DoubleRow

# dtype knobs: Q/K projections stay bf16 (fp8 there costs ~3e-2 rel err via
# exp amplification); the V projection and output projection run fp8
# DoubleRow (0.5 cyc/row) with weights prescaled by 32 - their errors get
# averaged by the attention weights / the k=256 contraction.
V_FP8 = False
CT_FP8 = False

QKV_DT = BF16
V_DT = FP8 if V_FP8 else BF16
CT_DT = FP8 if CT_FP8 else BF16
W_PRE = 32.0  # fp8 weight prescale (undone in the bias/output steps)

# packed column order of heads inside the [128, 4*128] score/exp tiles:
# heads 0,2 land on psum rows 0:64 (ct partitions 0:64 of their pt), heads
# 1,3 on rows 64:128; recip-broadcast needs even heads in cols 0:256.
HORDER = (0, 2, 1, 3)

PHASES = "ABC"  # debug switch: subset of phases to emit
BSUB = 3        # debug switch: phase-B depth (1=scores/exp, 2=+den/AV, 3=full)


def _build_nc(reps=1):
    nc = bacc.Bacc("TRN2", debug=False)

    xt_in = nc.dram_tensor("xt", [D, S], QKV_DT, kind="ExternalInput").ap()
    xtf_in = nc.dram_tensor("xtf", [D, S], V_DT, kind="ExternalInput").ap()
    wq_in = nc.dram_tensor("wq", [D, DPC], QKV_DT, kind="ExternalInput").ap()
    wk_in = nc.dram_tensor("wk", [D, DPC], QKV_DT, kind="ExternalInput").ap()
    wv_in = nc.dram_tensor("wv", [D, DPC], V_DT, kind="ExternalInput").ap()
    wo_in = nc.dram_tensor("wo", [DPC, D], CT_DT, kind="ExternalInput").ap()
    bq_in = nc.dram_tensor("bq2", [128, NPT], F32, kind="ExternalInput").ap()
    bk_in = nc.dram_tensor("bk2", [128, NPT], F32, kind="ExternalInput").ap()
    bv_in = nc.dram_tensor("bv2", [128, NPT], F32, kind="ExternalInput").ap()
    decay_in = nc.dram_tensor("decay", [128, 4 * 512], BF16, kind="ExternalInput").ap()
    out = nc.dram_tensor("out", [S, D], BF16, kind="ExternalOutput").ap()

    with tile.TileContext(nc) as tc:
        with (
            tc.tile_pool(name="singles", bufs=1) as singles,
            tc.tile_pool(name="work", bufs=2) as work,
            tc.tile_pool(name="small", bufs=3) as small,
            tc.tile_pool(name="psum", bufs=1, space="PSUM") as psum,
        ):
            # ---- persistent constants ----
            wq_sb = singles.tile([128, KT, DPC], QKV_DT)
            wk_sb = singles.tile([128, KT, DPC], QKV_DT)
            wv_sb = singles.tile([128, KT, DPC], V_DT)
            wo_sb = singles.tile([128, NPT, D], CT_DT)
            nc.sync.dma_start(wq_sb, wq_in.rearrange("(kt p) m -> p kt m", p=128))
            nc.sync.dma_start(wk_sb, wk_in.rearrange("(kt p) m -> p kt m", p=128))
            nc.sync.dma_start(wv_sb, wv_in.rearrange("(kt p) m -> p kt m", p=128))
            nc.sync.dma_start(wo_sb, wo_in.rearrange("(pt p) n -> p pt n", p=128))

            bq_sb = singles.tile([128, NPT], F32)
            bk_sb = singles.tile([128, NPT], F32)
            bv_sb = singles.tile([128, NPT], F32)
            nc.sync.dma_start(bq_sb, bq_in)
            nc.sync.dma_start(bk_sb, bk_in)
            nc.sync.dma_start(bv_sb, bv_in)

            decay_sb = singles.tile([128, 4, 512], BF16)
            nc.sync.dma_start(decay_sb, decay_in.rearrange("p (k c) -> p k c", k=4))

            ones_col = singles.tile([128, 1], BF16)   # den matmul lhsT
            nc.vector.memset(ones_col, 1.0)
            ones_row = singles.tile([1, 64], BF16)    # recip broadcast lhsT
            nc.vector.memset(ones_row, 1.0)

            env = dict(
                xt_in=xt_in, xtf_in=xtf_in, out=out,
                wq_sb=wq_sb, wk_sb=wk_sb, wv_sb=wv_sb, wo_sb=wo_sb,
                bq_sb=bq_sb, bk_sb=bk_sb, bv_sb=bv_sb,
                decay_sb=decay_sb, ones_col=ones_col, ones_row=ones_row,
            )
            for _rep in range(reps):
                _phases(nc, work, small, psum, env)

    nc.compile()
    return nc


def _phases(nc, work, small, psum, env):
    xt_in = env["xt_in"]; xtf_in = env["xtf_in"]; out = env["out"]
    wq_sb = env["wq_sb"]; wk_sb = env["wk_sb"]; wv_sb = env["wv_sb"]
    wo_sb = env["wo_sb"]
    bq_sb = env["bq_sb"]; bk_sb = env["bk_sb"]; bv_sb = env["bv_sb"]
    decay_sb = env["decay_sb"]; ones_col = env["ones_col"]; ones_row = env["ones_row"]

    vdiv = (1.0 / W_PRE) if V_FP8 else 1.0
    oscale = (1.0 / W_PRE) if CT_FP8 else 1.0

    xt = work.tile([128, KT, S], QKV_DT, tag="xt", bufs=1)
    xtf = (work.tile([128, KT, S], V_DT, tag="xtf", bufs=1) if V_FP8 else None)
    qt = work.tile([128, NPT, S], BF16, tag="qt", bufs=2)
    kt_sb = work.tile([128, NPT, S], BF16, tag="kt", bufs=2)
    # odd heads' features partition-shifted to 0:64 so every scores matmul
    # shares PE tile row 0 (same-bank matmuls must share tile geometry)
    qlo = work.tile([64, NPT, S], BF16, tag="qlo", bufs=2)
    klo = work.tile([64, NPT, S], BF16, tag="klo", bufs=2)
    vaug = work.tile([128, NST, DPC], BF16, tag="vaug", bufs=2)
    ct = work.tile([128, NPT, S], CT_DT, tag="ct", bufs=2)

    xt_src = xt_in.rearrange("(kt p) s -> p kt s", p=128)
    xtf_src = xtf_in.rearrange("(kt p) s -> p kt s", p=128)
    env["_qt"], env["_kt"], env["_vaug"], env["_ct"] = qt, kt_sb, vaug, ct

    # ---- phase A: project Q^T, K^T (feature-major) and V (natural) ----
    for c in range(4):
        cs = slice(c * 512, (c + 1) * 512)
        nc.sync.dma_start(xt[:, :, cs], xt_src[:, :, cs])
        if V_FP8:
            nc.sync.dma_start(xtf[:, :, cs], xtf_src[:, :, cs])
        for pt in range(NPT):
            ptb = slice(pt * 128, (pt + 1) * 128)
            for (w_sb, dst, s1, s2) in (
                (wq_sb, qt, 0.125, bq_sb),
                (wk_sb, kt_sb, 1.0, bk_sb),
                (wv_sb, None, vdiv, bv_sb),
            ):
                ps_p = psum.tile([128, 512], F32, tag="big", bufs=2)
                if dst is None and V_FP8:
                    for k in range(0, KT, 2):
                        nc.tensor.matmul(
                            ps_p,
                            lhsT=w_sb[:, k : k + 2, ptb],
                            rhs=xtf[:, k : k + 2, cs],
                            start=(k == 0),
                            stop=(k == KT - 2),
                            perf_mode=DR,
                        )
                else:
                    for k in range(KT):
                        nc.tensor.matmul(
                            ps_p,
                            lhsT=w_sb[:, k, ptb],
                            rhs=xt[:, k, cs],
                            start=(k == 0),
                            stop=(k == KT - 1),
                        )
                if dst is not None:
                    nc.vector.tensor_scalar(
                        dst[:, pt, cs], ps_p, s1, s2[:, pt : pt + 1],
                        ALU.mult, ALU.add,
                    )
                    lo = qlo if dst is qt else klo
                    nc.sync.dma_start(lo[0:64, pt, cs], dst[64:128, pt, cs])
                else:
                    # V: bias (GPSIMD cannot read PSUM, so DVE), then
                    # DMA-transpose into natural layout
                    vt_tmp = small.tile([128, 512], BF16, tag="vt", bufs=2)
                    nc.vector.tensor_scalar(
                        vt_tmp, ps_p, s1, s2[:, pt : pt + 1],
                        ALU.mult, ALU.add,
                    )
                    nc.sync.dma_start_transpose(
                        vaug[:, 4 * c : 4 * c + 4, pt * 128 : (pt + 1) * 128],
                        vt_tmp,
                    )

    # ---- phase B: banded attention per 128-row s-tile ----
    # Three 128-aligned t-tiles tc in {st-1, st, st+1} cover every query row
    # with at least +-128 of context (decay beyond that < 1e-40).  All score
    # matmuls read feature rows 0:64 (odd heads via the shifted qlo/klo), and
    # all AV matmuls are k=128 vaug blocks, so every PSUM bank sees a single
    # PE tile geometry.  Edge tiles use a clamped (valid-memory) tc with an
    # all-zero decay kind.
    for st in range(NST) if "B" in PHASES else ():
        s0 = st * 128
        ss = slice(s0, s0 + 128)
        ps_av = psum.tile([128, 512], F32, tag="av", bufs=2)
        den = psum.tile([1, 512], F32, tag="den", bufs=1)
        exds = []
        for j in range(3):
            tc = st + j - 1
            if 0 <= tc < NST:
                dk = j
            else:
                tc, dk = st, 3  # invalid neighbor: zero decay kills it
            ts_ = slice(tc * 128, (tc + 1) * 128)
            ps_sc = psum.tile([128, 512], F32, tag="sc", bufs=2)
            for ci, h in enumerate(HORDER):
                pt, odd = h // 2, h % 2
                kk = klo if odd else kt_sb
                qq = qlo if odd else qt
                nc.tensor.matmul(
                    ps_sc[:, ci * 128 : (ci + 1) * 128],
                    lhsT=kk[0:64, pt, ts_],
                    rhs=qq[0:64, pt, ss],
                    start=(ci == 0),
                    stop=(ci == 3),
                )
            ex = small.tile([128, 512], BF16, tag="ex", bufs=3)
            nc.scalar.activation(ex, ps_sc, AF.Exp)
            exd = small.tile([128, 512], BF16, tag="exd", bufs=4)
            eng = nc.gpsimd if j == 1 and st % 2 == 0 else nc.vector
            eng.tensor_mul(exd, ex, decay_sb[:, dk, :])
            exds.append(exd)
            nc.tensor.matmul(
                den, lhsT=ones_col, rhs=exd, start=(j == 0), stop=(j == 2)
            )
            # AV: k=128 aligned vaug blocks; one accumulation group per
            # partition-half of the packed bank (sim's group-check mis-maps
            # offset-64 psum APs, so those skip the sim-only check)
            for ci, h in enumerate(HORDER):
                pt, r0 = h // 2, 64 * (h % 2)
                nc.tensor.matmul(
                    ps_av[r0 : r0 + 64, pt * 128 : (pt + 1) * 128],
                    lhsT=vaug[:, tc, h * 64 : (h + 1) * 64],
                    rhs=exd[:, ci * 128 : (ci + 1) * 128],
                    start=(j == 0 and ci in (0, 2)),
                    stop=(j == 2 and ci in (1, 3)),
                    skip_group_check=(r0 == 64),
                )
        rec = small.tile([1, 512], BF16, tag="rec", bufs=2)
        with nc.allow_low_precision(reason="normalizer tolerates bf16"):
            nc.vector.reciprocal(rec, den)
        rb = psum.tile([128, 512], F32, tag="rb", bufs=1)
        nc.tensor.matmul(
            rb[0:64, 0:256], lhsT=ones_row, rhs=rec[0:1, 0:256],
            start=True, stop=True,
        )
        nc.tensor.matmul(
            rb[64:128, 0:256], lhsT=ones_row, rhs=rec[0:1, 256:512],
            start=True, stop=True, skip_group_check=True,
        )
        # DVE may read only one PSUM operand: stage rb in SBUF via ACT
        rb_sb = small.tile([128, 256], BF16, tag="rbs", bufs=2)
        nc.scalar.activation(rb_sb, rb[:, 0:256], AF.Copy)
        nc.vector.tensor_mul(
            ct[:, :, ss],
            ps_av[:, 0:256].rearrange("p (a b) -> p a b", a=NPT),
            rb_sb.rearrange("p (a b) -> p a b", a=NPT),
        )

    # ---- phase C: output projection (row-sharded Wo -> partial sums) ----
    for c2 in range(NST) if "C" in PHASES else ():
        c2s = slice(c2 * 128, (c2 + 1) * 128)
        osb = small.tile([128, 2, 512], BF16, tag="osb", bufs=2)
        for n in range(2):
            ps_o = psum.tile([128, 512], F32, tag="big", bufs=2)
            ns = slice(n * 512, (n + 1) * 512)
            if CT_FP8:
                nc.tensor.matmul(
                    ps_o,
                    lhsT=ct[:, :, c2s],
                    rhs=wo_sb[:, :, ns],
                    start=True,
                    stop=True,
                    perf_mode=DR,
                )
            else:
                for pt in range(NPT):
                    nc.tensor.matmul(
                        ps_o,
                        lhsT=ct[:, pt, c2s],
                        rhs=wo_sb[:, pt, ns],
                        start=(pt == 0),
                        stop=(pt == NPT - 1),
                    )
            # balance the PSUM->SBUF copies between ACT and DVE
            if (c2 * 2 + n) % 4 == 3:
                nc.vector.tensor_scalar_mul(osb[:, n, :], ps_o, oscale)
            else:
                nc.scalar.activation(osb[:, n, :], ps_o, AF.Copy, scale=oscale)
        nc.sync.dma_start(out[c2s, :], osb)


def _make_in_maps(x, Wq, bq, Wk, bk, Wv, bv, Wo, bo, slopes):
    """Host-side sharding + layout prep: core id = b*4 + g."""
    qkv_np = mybir.dt.np(QKV_DT)
    v_np = mybir.dt.np(V_DT)
    ct_np = mybir.dt.np(CT_DT)
    vpre = W_PRE if V_FP8 else 1.0
    opre = W_PRE if CT_FP8 else 1.0

    p = np.arange(128)[:, None].astype(np.float64)
    f = np.arange(128)[None, :].astype(np.float64)
    dists = [
        np.abs(p - 128.0 - f),  # 0: t-tile at offset -1
        np.abs(p - f),          # 1: offset 0
        np.abs(p + 128.0 - f),  # 2: offset +1
        np.abs(p - f),          # 3: zero kind (edge-invalid tiles)
    ]
    masks = [None, None, None, p < -1]  # kind 3: all-False mask -> zeros

    in_maps = []
    for b in range(B):
        for g in range(NCORES // B):
            cols = slice(g * DPC, (g + 1) * DPC)
            sp = np.log1p(np.exp(slopes[g * HPC : (g + 1) * HPC].astype(np.float64)))
            # decay [128, kind(4), head-block(HORDER), 128]
            dec = np.zeros((128, 4, HPC, 128), np.float64)
            for k in range(4):
                for ci, h in enumerate(HORDER):
                    d = np.exp(-sp[h] * dists[k])
                    if masks[k] is not None:
                        d = d * masks[k]
                    dec[:, k, ci, :] = d
            in_maps.append(
                {
                    "xt": np.ascontiguousarray(x[b].T).astype(qkv_np),
                    "xtf": np.ascontiguousarray(x[b].T).astype(v_np),
                    "wq": Wq[:, cols].astype(qkv_np),
                    "wk": Wk[:, cols].astype(qkv_np),
                    "wv": (Wv[:, cols] * vpre).astype(v_np),
                    "wo": (Wo[cols, :] * opre).astype(ct_np),
                    "bq2": np.ascontiguousarray(
                        (bq[cols] / 8.0).reshape(NPT, 128).T
                    ).astype(np.float32),
                    "bk2": np.ascontiguousarray(
                        bk[cols].reshape(NPT, 128).T
                    ).astype(np.float32),
                    "bv2": np.ascontiguousarray(
                        bv[cols].reshape(NPT, 128).T
                    ).astype(np.float32),
                    "decay": np.ascontiguousarray(
                        dec.reshape(128, 4 * 512)
                    ).astype(ml_dtypes.bfloat16),
                }
            )
    return in_maps


_NC_CACHE = None


def _get_nc():
    global _NC_CACHE
    if _NC_CACHE is None:
        _NC_CACHE = _build_nc()
    return _NC_CACHE


def kernel(x, Wq, bq, Wk, bk, Wv, bv, Wo, bo, slopes, **run_kwargs):
    args = [np.asarray(a, dtype=np.float32) for a in (x, Wq, bq, Wk, bk, Wv, bv, Wo, bo, slopes)]
    x, Wq, bq, Wk, bk, Wv, bv, Wo, bo, slopes = args
    nc = _get_nc()
    in_maps = _make_in_maps(x, Wq, bq, Wk, bk, Wv, bv, Wo, bo, slopes)
    res = run_bass_kernel_spmd(nc, in_maps, core_ids=list(range(NCORES)), **run_kwargs)
    parts = [r["out"] for r in res.results]
    out = np.empty((B, S, D), np.float32)
    for b in range(B):
        acc = parts[b * 4].astype(np.float32)
        for g in range(1, NCORES // B):
            acc = acc + parts[b * 4 + g].astype(np.float32)
        out[b] = acc + bo[None, :]
    if run_kwargs:
        kernel.last_results = res
    return out
